# revision 1
# baseline (speedup 1.0000x reference)
"""GQA attention (B=4, L=1024, D=4096, 32 Q heads / 8 KV heads, head_dim=128,
traditional RoPE, causal mask) on 8 TRN2 NeuronCores.

Sharding: tensor-parallel over heads. Core c owns Q heads {c, c+8, c+16, c+24}
(all map to KV head c) - each core needs exactly one KV head. wq/wk/wv
column-sharded, wo row-sharded, x replicated. Each core computes a partial
output through wo; the host sums the 8 partials (and transposes: the kernel
writes out^T [DIM, T] in bf16).

v2 changes vs baseline:
- RoPE with zero tensor-engine work: wq/wk columns are host-permuted so the
  rotation pairs (2i, 2i+1) land at partitions (i, 64+i). The pair-swap then
  becomes two half-partition DVE adds against contiguous halves - no 128x128
  permutation matmuls, no extra PSUM bank, no head-of-line stalls.
- Startup: weights stream in per-d-slice DMAs interleaved with the first x
  chunk, so the first matmul issues ~1us in instead of ~31us.
- Attention uses 256-token q chunks (finer causal skip: 10 vs 12 block-equivs
  per head) and 4 q-head scores per PSUM group tile, exp'd in one wide
  ACT instruction spanning banks.
- Output projection keeps wo stationary and streams attn, accumulating both
  512-token halves per LDWEIGHTS; output is written transposed in bf16,
  halving the write traffic.
"""

import numpy as np
import ml_dtypes
from contextlib import ExitStack

import concourse.bass as bass
import concourse.mybir as mybir
import concourse.tile as tile
from concourse import bacc
from concourse.bass_utils import run_bass_kernel_spmd

DIM = 4096
N_HEADS = 32
N_KV = 8
DH = 128
B, L = 4, 1024
NCORES = 8
HPC = N_HEADS // NCORES  # 4 q-heads per core
T = B * L  # 4096 tokens total
SCALE = DH ** -0.5
ROPE_BASE = 10000.0
NDT = DIM // 128  # 32 contraction tiles

BF = mybir.dt.bfloat16
F32 = mybir.dt.float32
NPBF = ml_dtypes.bfloat16

QC4 = L // 256  # 4 q-chunks of 256 per batch
KT = L // 128   # 8 k tiles of 128 per batch

TRACE = False
LAST_RESULT = [None]


def _check_mask(mask):
    """Verify the mask is the binary causal mask this kernel is specialized
    to, and return the 8 diagonal 128x256 keep-patterns [128, 8, 256]."""
    m = np.asarray(mask)
    assert m.shape == (L, L)
    assert np.all((m == 0.0) | (m <= -1e8)), "kernel assumes binary additive mask"
    keep = (m == 0.0)  # [q, k]
    dmask = np.zeros((128, KT, 256), dtype=np.float32)
    for qc in range(QC4):
        qs = slice(qc * 256, (qc + 1) * 256)
        for kt in range(KT):
            ks = slice(kt * 128, (kt + 1) * 128)
            blk = keep[qs, ks]  # [256 q, 128 k]
            if kt > 2 * qc + 1:
                assert not blk.any(), f"block ({kt},{qc}) expected fully masked"
            elif kt < 2 * qc:
                assert blk.all(), f"block ({kt},{qc}) expected free"
            else:
                dmask[:, kt, :] = blk.T  # [128 k, 256 q]
    return dmask


def _build():
    nc = bacc.Bacc(
        "TRN2", target_bir_lowering=False, debug=False, num_devices=NCORES
    )

    xT = nc.dram_tensor("xT", [DIM, T], BF, kind="ExternalInput").ap()
    # all QKV weights in one tensor, d-major: [:, d] = [wv_d | wk_d | wq_d]
    # (128+128+512 cols) so a 4-d chunk loads as ONE large dma_start
    wall = nc.dram_tensor("wall", [128, NDT * 768], BF, kind="ExternalInput").ap()
    wo = nc.dram_tensor("wo", [128, HPC * DIM], BF, kind="ExternalInput").ap()
    cosq = nc.dram_tensor("cosq", [128, 2 * HPC * 512], BF, kind="ExternalInput").ap()
    sinq = nc.dram_tensor("sinq", [128, 2 * HPC * 512], BF, kind="ExternalInput").ap()
    cosk = nc.dram_tensor("cosk", [128, L], BF, kind="ExternalInput").ap()
    sink = nc.dram_tensor("sink", [128, L], BF, kind="ExternalInput").ap()
    ident = nc.dram_tensor("ident", [DH, DH], BF, kind="ExternalInput").ap()
    dmask = nc.dram_tensor("dmask", [128, KT * 256], BF, kind="ExternalInput").ap()
    out = nc.dram_tensor("out", [DIM, T], BF, kind="ExternalOutput").ap()

    xT_r = xT.rearrange("(dt p) t -> dt p t", p=128)  # [32, 128, 4096]
    wall_r = wall.rearrange("p (dt m) -> p dt m", dt=NDT)  # [128, 32, 768]

    with tile.TileContext(nc) as tc, ExitStack() as ctx:
        persist = ctx.enter_context(tc.tile_pool(name="persist", bufs=1))
        qt_pool = ctx.enter_context(tc.tile_pool(name="qt", bufs=B))
        kt_pool = ctx.enter_context(tc.tile_pool(name="kt", bufs=B))
        v_pool = ctx.enter_context(tc.tile_pool(name="v", bufs=B))
        wo_p = ctx.enter_context(tc.tile_pool(name="wo_p", bufs=1))

        ones_sb = persist.tile([128, 128], BF)
        nc.vector.memset(ones_sb, 1.0)
        cosq_sb = persist.tile([128, 2, HPC, 512], BF)
        sinq_sb = persist.tile([128, 2, HPC, 512], BF)
        cosk_sb = persist.tile([128, L], BF)
        sink_sb = persist.tile([128, L], BF)
        idn_sb = persist.tile([DH, DH], BF)
        dmsk_sb = persist.tile([128, KT, 256], BF)
        wo_sb = wo_p.tile([128, HPC, DIM], BF)

        qt_all = [None] * B  # [128 dh, HPC, 1024] rope'd q, halves layout
        kt_all = [None] * B  # [128 dh, 1024]
        v_t = [None] * B     # [128 t, KT, 128 dh]

        # ---------------- Stage A: QKV projection + RoPE ----------------
        with tc.tile_pool(name="wA", bufs=1) as wA, \
             tc.tile_pool(name="xp", bufs=8) as xp, \
             tc.tile_pool(name="evq", bufs=2) as evq, \
             tc.tile_pool(name="evs", bufs=2) as evs, \
             tc.tile_pool(name="rtmp", bufs=1) as rtmp, \
             tc.tile_pool(name="psA", bufs=1, space="PSUM") as psA, \
             tc.tile_pool(name="psT", bufs=2, space="PSUM") as psT:

            wall_sb = wA.tile([128, NDT, 768], BF)

            def _wv(d):
                return wall_sb[:, d, 0:DH]

            def _wk(d):
                return wall_sb[:, d, DH:2 * DH]

            def _wq(d, h):
                return wall_sb[:, d, 2 * DH + h * DH:2 * DH + (h + 1) * DH]

            # weight chunks of 4 d-slices, one dma_start each; two issued
            # before the loop so the stream stays ahead of compute
            def _wchunk(k):
                # scalar ring: weight chunks stream concurrently with the
                # x tiles on the sync ring instead of queuing behind them
                dsl = slice(4 * k, 4 * k + 4)
                nc.scalar.dma_start(out=wall_sb[:, dsl], in_=wall_r[:, dsl])

            nc.scalar.dma_start(out=wall_sb[:, 0:1], in_=wall_r[:, 0:1])
            nc.scalar.dma_start(out=wall_sb[:, 1:4], in_=wall_r[:, 1:4])

            nxt_prefetch = {}
            for tci in range(T // 512):  # 8 chunks of 512 tokens
                b, half = tci // 2, tci % 2
                lsl = slice(half * 512, (half + 1) * 512)
                if half == 0:
                    qt_all[b] = qt_pool.tile([128, HPC, L], BF, name="qtile")
                    kt_all[b] = kt_pool.tile([128, L], BF, name="ktile")
                    v_t[b] = v_pool.tile([128, KT, DH], BF, name="vtile")

                # bank order: v(0), k(1), q(2..5)
                ps_v = psA.tile([128, 512], F32, name="psv")
                ps_k = psA.tile([128, 512], F32, name="psk")
                ps_q = psA.tile([128, HPC, 512], F32, name="psq")

                prefetched = {}
                if tci == 1:
                    prefetched.update(nxt_prefetch)
                    nxt_prefetch.clear()
                if tci == 0:
                    # x prefetch interleaved with the second weight chunk so
                    # the first matmul's critical path is chunk0 + xt0 only
                    for d in range(2):
                        xt = xp.tile([128, 512], BF)
                        nc.sync.dma_start(out=xt, in_=xT_r[d, :, 0:512])
                        prefetched[d] = xt
                    _wchunk(1)
                    for d in range(2, 4):
                        xt = xp.tile([128, 512], BF)
                        nc.sync.dma_start(out=xt, in_=xT_r[d, :, 0:512])
                        prefetched[d] = xt
                    # HAM warmup: keep the PE busy while DMAs land so the
                    # clock gate opens before the real stream begins
                    for wu in range(40):
                        nc.tensor.matmul(
                            ps_q[:, 0, 0:128], ones_sb, ones_sb,
                            start=True, stop=True,
                        )

                for d in range(NDT):
                    if tci == 0:
                        if d % 4 == 1 and d // 4 + 2 < 8:
                            _wchunk(d // 4 + 2)
                        if d == 8:
                            # small constants early (ident gates the tci-0
                            # v-transposes); DMAs must be EMITTED before
                            # their first consumer for dep tracking
                            nc.scalar.dma_start(out=idn_sb, in_=ident)
                            nc.scalar.dma_start(
                                out=dmsk_sb,
                                in_=dmask.rearrange("p (k t) -> p k t", k=KT),
                            )
                        if d == 24:
                            # big cos/sin tables after the weight storm
                            nc.scalar.dma_start(
                                out=cosq_sb,
                                in_=cosq.rearrange(
                                    "p (a h t) -> p a h t", a=2, h=HPC
                                ),
                            )
                            nc.scalar.dma_start(
                                out=sinq_sb,
                                in_=sinq.rearrange(
                                    "p (a h t) -> p a h t", a=2, h=HPC
                                ),
                            )
                            nc.scalar.dma_start(out=cosk_sb, in_=cosk)
                            nc.scalar.dma_start(out=sink_sb, in_=sink)
                    if tci == 0 and d == 28:
                        # prefetch the next chunk's first x tiles across the
                        # tci boundary so the x stream never drains
                        for dn in range(3):
                            xtn = xp.tile([128, 512], BF)
                            nc.sync.dma_start(
                                out=xtn, in_=xT_r[dn, :, 512:1024]
                            )
                            nxt_prefetch[dn] = xtn
                    if tci == 1 and d == 16:
                        nc.scalar.dma_start(
                            out=wo_sb, in_=wo.rearrange("p (h n) -> p h n", h=HPC)
                        )
                    if d in prefetched:
                        xt = prefetched.pop(d)
                    else:
                        xt = xp.tile([128, 512], BF)
                        nc.sync.dma_start(
                            out=xt, in_=xT_r[d, :, tci * 512:(tci + 1) * 512]
                        )
                    st, sp = d == 0, d == NDT - 1
                    nc.tensor.matmul(ps_v, _wv(d), xt, start=st, stop=sp)
                    nc.tensor.matmul(ps_k, _wk(d), xt, start=st, stop=sp)
                    for h in range(HPC):
                        nc.tensor.matmul(
                            ps_q[:, h], _wq(d, h), xt, start=st, stop=sp,
                        )

                # --- tail: evacuate + v-transpose + RoPE (no PE rope work) ---
                # evac split across ACT (vraw, q01) and DVE (kraw, q23) so
                # PSUM banks free in a staggered pattern for the next tci
                vraw = evs.tile([128, 512], BF, name="vraw")
                nc.scalar.copy(vraw, ps_v)
                for s in range(4):
                    ps_t = psT.tile([128, 128], BF, name="pstr")
                    nc.tensor.transpose(ps_t, vraw[:, s * 128:(s + 1) * 128], idn_sb)
                    nc.vector.tensor_copy(v_t[b][:, half * 4 + s], ps_t)

                kraw = evs.tile([128, 512], BF, name="kraw")
                nc.vector.tensor_copy(kraw, ps_k)
                qraw = evq.tile([128, HPC, 512], BF, name="qraw")
                nc.scalar.copy(qraw[:, 0:2], ps_q[:, 0:2])
                nc.vector.tensor_copy(qraw[:, 2:4], ps_q[:, 2:4])

                # RoPE: dst = raw*cos + swap64(raw*sinSw)  (halves layout).
                # TT inputs must share a base partition (NCC_IBIR297), so the
                # 64-half swap goes through single-input copies.
                u_k = rtmp.tile([128, 512], BF, name="uk")
                t_k = rtmp.tile([128, 512], BF, name="tk")
                usw_k = rtmp.tile([128, 512], BF, name="uswk")
                nc.vector.tensor_mul(u_k, kraw, sink_sb[:, lsl])
                nc.vector.tensor_mul(t_k, kraw, cosk_sb[:, lsl])
                nc.vector.tensor_copy(usw_k[0:64], u_k[64:128])
                nc.vector.tensor_copy(usw_k[64:128], u_k[0:64])
                nc.vector.tensor_add(kt_all[b][:, lsl], t_k, usw_k)

                u_q = rtmp.tile([128, HPC, 512], BF, name="uq")
                t_q = rtmp.tile([128, HPC, 512], BF, name="tq")
                usw_q = rtmp.tile([128, HPC, 512], BF, name="uswq")
                nc.vector.tensor_mul(u_q, qraw, sinq_sb[:, half])
                nc.vector.tensor_mul(t_q, qraw, cosq_sb[:, half])
                nc.vector.tensor_copy(usw_q[0:64], u_q[64:128])
                nc.vector.tensor_copy(usw_q[64:128], u_q[0:64])
                nc.vector.tensor_add(qt_all[b][:, :, lsl], t_q, usw_q)

        # ---------------- Stages B + C, per batch ----------------
        # one pool set for all batches: no pool open/close barriers between
        # B(b) -> C(b) -> B(b+1); C's output tiles share the psS tag (same
        # 2-bank slot size), so the PE flows straight across stages.
        attn_pool = ctx.enter_context(tc.tile_pool(name="attn", bufs=2))
        ep = ctx.enter_context(tc.tile_pool(name="ep", bufs=4))
        rcp = ctx.enter_context(tc.tile_pool(name="rcp", bufs=2))
        oev = ctx.enter_context(tc.tile_pool(name="oev", bufs=4))
        psS = ctx.enter_context(tc.tile_pool(name="psS", bufs=2, space="PSUM"))
        psPV = ctx.enter_context(tc.tile_pool(name="psPV", bufs=2, space="PSUM"))
        psSum = ctx.enter_context(tc.tile_pool(name="psSum", bufs=2, space="PSUM"))
        attn_all = [None] * B

        for b in range(B):
            attn_all[b] = attn_pool.tile([128, HPC, L], BF, name="atile")

            # ---- B(b): attention, software-pipelined one group ahead so
            # the PE always has the next group's score matmuls queued while
            # ACT runs exp on the previous group ----
            pvs = {}  # (h, qc) -> (pv tile, sum tile); separate banks: a
            # start=True matmul clears its WHOLE bank, so pv and sum
            # must not share one
            pending = [None]

            def flush_pv(item):
                h, qc, g, e_g, e4, first, last = item
                if first:
                    pvs[(h, qc)] = (
                        psPV.tile([128, 256], F32, name="pspv",
                                  padded_shape=[128, 512]),
                        psSum.tile([128, 256], F32, name="pssum",
                                   padded_shape=[128, 512]),
                    )
                pv, psum = pvs[(h, qc)]
                for i, kt in enumerate(g):
                    st = first and i == 0
                    sp = last and i == len(g) - 1
                    nc.tensor.matmul(
                        pv, v_t[b][:, kt], e_g[:, i],
                        start=st, stop=sp,
                    )
                nc.tensor.matmul(
                    psum, ones_sb, e4, start=first, stop=last,
                )
                if last:
                    q_sl = slice(qc * 256, (qc + 1) * 256)
                    recip = rcp.tile([128, 256], F32, name="recip")
                    nc.vector.reciprocal_approx_fast(recip, psum)
                    nc.vector.tensor_mul(
                        attn_all[b][:, h, q_sl], pv, recip
                    )
                    del pvs[(h, qc)]

            for h in range(HPC):
                # sequence of (qc, group, first_of_qc, last_of_qc)
                seq = []
                for qc in range(QC4):
                    kts = list(range(2 * qc + 2))
                    groups = [g for g in (kts[0:4], kts[4:]) if g]
                    for gi, g in enumerate(groups):
                        seq.append((qc, g, gi == 0, gi == len(groups) - 1))

                for qc, g, first, last in seq:
                    q_sl = slice(qc * 256, (qc + 1) * 256)
                    s = len(g)
                    ps_s = psS.tile(
                        [128, s, 256], F32, name="pss",
                        padded_shape=[128, 4, 256],
                    )
                    for i, kt in enumerate(g):
                        nc.tensor.matmul(
                            ps_s[:, i],
                            kt_all[b][:, kt * 128:(kt + 1) * 128],
                            qt_all[b][:, h, q_sl],
                            start=True, stop=True,
                        )
                    e_g = ep.tile([128, s, 256], BF, name="etile")
                    nc.scalar.activation(
                        e_g, ps_s,
                        mybir.ActivationFunctionType.Exp,
                        scale=SCALE,
                    )
                    if last:
                        nc.vector.tensor_mul(
                            e_g[:, s - 2:s], e_g[:, s - 2:s],
                            dmsk_sb[:, 2 * qc:2 * qc + 2],
                        )
                    # tree pre-add on DVE so the softmax denominator needs
                    # only ONE ones-matmul per group
                    e2 = ep.tile([128, s // 2, 256], BF, name="e2tile")
                    nc.vector.tensor_add(
                        e2, e_g[:, 0:s:2], e_g[:, 1:s:2]
                    )
                    if s == 4:
                        e4 = ep.tile([128, 256], BF, name="e4tile")
                        nc.vector.tensor_add(e4, e2[:, 0], e2[:, 1])
                    else:
                        e4 = e2[:, 0]
                    if pending[0] is not None:
                        flush_pv(pending[0])
                    pending[0] = (h, qc, g, e_g, e4, first, last)
            flush_pv(pending[0])

            # ---- C(b): output projection, wo stationary, out^T ----
            if True:
                for nb in range(DIM // 128):  # 32 blocks of 128 output cols
                    if nb < 30:
                        ps_c = psS.tile([128, 2, 512], F32, name="pss")
                        halves = [ps_c[:, 0], ps_c[:, 1]]
                    else:
                        # last two blocks borrow the pv/sum banks so psS
                        # frees early for the next batch's score groups
                        ps_c = None
                        halves = [
                            psPV.tile([128, 512], F32, name="pspv"),
                            psSum.tile([128, 512], F32, name="pssum"),
                        ]
                    for h in range(HPC):
                        for t2 in range(2):
                            nc.tensor.matmul(
                                halves[t2],
                                wo_sb[:, h, nb * 128:(nb + 1) * 128],
                                attn_all[b][:, h, t2 * 512:(t2 + 1) * 512],
                                start=(h == 0), stop=(h == HPC - 1),
                            )
                    o_sb = oev.tile([128, 1024], BF, name="osb")
                    if ps_c is not None:
                        if nb % 2 == 0:
                            nc.vector.tensor_copy(o_sb, ps_c)
                        else:
                            nc.scalar.copy(o_sb, ps_c)
                    else:
                        nc.vector.tensor_copy(o_sb[:, 0:512], halves[0])
                        nc.scalar.copy(o_sb[:, 512:1024], halves[1])
                    nc.sync.dma_start(
                        out=out[nb * 128:(nb + 1) * 128, b * L:(b + 1) * L],
                        in_=o_sb,
                    )

    nc.finalize()
    return nc


def _host_tables():
    """cos/sin tables in the halves layout: row i (i<64) = even dim 2i,
    row 64+i = odd dim 2i+1. u = raw*sinSw; dst_lo = t1_lo + u_hi needs
    sinSw = [+sin; -sin]; cosH = [cos; cos]."""
    inv = ROPE_BASE ** (-np.arange(0, DH, 2, dtype=np.float64) / DH)  # [64]
    pos = np.arange(L, dtype=np.float64)
    ang = inv[:, None] * pos[None, :]  # [64, L]
    cosA, sinA = np.cos(ang), np.sin(ang)
    cosH = np.concatenate([cosA, cosA], axis=0)  # [128, L]
    sinSw = np.concatenate([sinA, -sinA], axis=0)  # [128, L]
    # q tables: [128, 2 halves, HPC, 512] with the same positional slice
    # repeated across heads
    cosq = np.empty((128, 2, HPC, 512), dtype=np.float64)
    sinq = np.empty((128, 2, HPC, 512), dtype=np.float64)
    for half in range(2):
        sl = slice(half * 512, (half + 1) * 512)
        cosq[:, half] = cosH[:, sl][:, None, :]
        sinq[:, half] = sinSw[:, sl][:, None, :]
    return (
        cosq.reshape(128, -1).astype(NPBF),
        sinq.reshape(128, -1).astype(NPBF),
        np.ascontiguousarray(cosH).astype(NPBF),
        np.ascontiguousarray(sinSw).astype(NPBF),
    )


def _ptile(w):
    # [K, M] -> partition-major [128, (K/128, M)] host pre-tiling
    k, m = w.shape
    return np.ascontiguousarray(
        w.reshape(k // 128, 128, m).transpose(1, 0, 2).reshape(128, -1)
    ).astype(NPBF)


def kernel(x, mask, wq, wk, wv, wo):
    x = np.asarray(x, dtype=np.float32)
    mask = np.asarray(mask, dtype=np.float32)
    wq = np.asarray(wq, dtype=np.float32)
    wk = np.asarray(wk, dtype=np.float32)
    wv = np.asarray(wv, dtype=np.float32)
    wo = np.asarray(wo, dtype=np.float32)

    dmask = _check_mask(mask)
    nc = _build()

    xT = np.ascontiguousarray(x.reshape(T, DIM).T).astype(NPBF)
    cosq, sinq, cosk, sink = _host_tables()
    idn = np.eye(DH, dtype=np.float32).astype(NPBF)
    dmask_in = np.ascontiguousarray(dmask.reshape(128, -1)).astype(NPBF)

    # halves permutation of the head_dim axis: even dims then odd dims
    perm = np.concatenate([np.arange(0, DH, 2), np.arange(1, DH, 2)])

    in_maps = []
    for c in range(NCORES):
        cols = np.concatenate(
            [np.arange(h * DH, (h + 1) * DH) for h in range(c, N_HEADS, N_KV)]
        )
        wq_c = wq[:, cols].reshape(DIM, HPC, DH)[:, :, perm].reshape(DIM, -1)
        wk_c = wk[:, c * DH:(c + 1) * DH][:, perm]
        wv_c = wv[:, c * DH:(c + 1) * DH]
        # d-major wall [128, NDT, 768]: [:, d] = [wv_d | wk_d | wq_d]
        pv_, pk_, pq_ = (
            _ptile(wv_c).reshape(128, NDT, DH),
            _ptile(wk_c).reshape(128, NDT, DH),
            _ptile(wq_c).reshape(128, NDT, HPC * DH),
        )
        wall = np.concatenate([pv_, pk_, pq_], axis=2).reshape(128, -1)
        in_maps.append({
            "xT": xT,
            "wall": np.ascontiguousarray(wall),
            "wo": _ptile(wo[cols, :]),
            "cosq": cosq,
            "sinq": sinq,
            "cosk": cosk,
            "sink": sink,
            "ident": idn,
            "dmask": dmask_in,
        })

    res = run_bass_kernel_spmd(
        nc, in_maps, core_ids=list(range(NCORES)), trace=TRACE
    )
    LAST_RESULT[0] = res
    outs = res.results
    total = np.zeros((DIM, T), dtype=np.float32)
    for c in range(NCORES):
        total += np.asarray(outs[c]["out"], dtype=np.float32)
    return np.ascontiguousarray(total.T).reshape(B, L, DIM)

